# revision 10
# baseline (speedup 1.0000x reference)
"""MoE (B=2048, D=1024, H=4096, E=8, top-2) — Trainium2 Bass kernel, 8 NeuronCores.

Strategy (expert-parallel, sparse token routing):
  * Host: gating (x @ Wg + bg, top-2, softmax) — 0.01% of the FLOPs. The
    token->expert routing IS the sharding step: core e receives the tokens
    assigned to expert e (gathered + padded to a common capacity C).
  * Device (per core e): outT = (gelu(x_e @ w1[e] + b1[e]) @ w2[e] + b2[e]).T
    computed in bf16 with fp32 PSUM accumulation, fully transposed layout so
    no on-device transposes are needed.
  * Host: weighted scatter-combine of the two expert outputs per token.

The dense reference computes all 8 experts per token; only the top-2 survive
the gates-weighted combine, so routing does 4x less matmul work.
"""

import json
import math

import numpy as np
from ml_dtypes import bfloat16

B, D, H, E, TOP_K = 2048, 1024, 4096, 8, 2
NCORES = 8
DBLK, HBLK = D // 128, H // 128  # 8, 32

TRACE = False  # test.py sets this to capture an NTFF profile / exec_time_ns
LAST_RESULTS = {}  # test.py reads exec_time_ns etc. from here


def _gate(x, Wg, bg):
    """Mirror of the reference gating math in numpy float32."""
    logits = (x @ Wg + bg).astype(np.float32)  # [B, E]
    rows = np.arange(B)
    i1 = np.argmax(logits, axis=1)
    v1 = logits[rows, i1]
    masked = logits.copy()
    masked[rows, i1] = -np.inf
    i2 = np.argmax(masked, axis=1)
    v2 = masked[rows, i2]
    # softmax over the top-2 values (v1 >= v2)
    e2 = np.exp((v2 - v1).astype(np.float32))
    denom = (np.float32(1.0) + e2).astype(np.float32)
    g1 = (np.float32(1.0) / denom).astype(np.float32)
    g2 = (e2 / denom).astype(np.float32)
    gates = np.zeros((B, E), np.float32)
    gates[rows, i1] = g1
    gates[rows, i2] = g2
    top_i = np.stack([i1, i2], axis=1).astype(np.int32)

    # load-balance aux loss
    def _cv(v):
        return np.std(v, ddof=1).astype(np.float32) / (
            np.mean(v, dtype=np.float32) + np.float32(1e-6)
        )

    importance = gates.sum(axis=0, dtype=np.float32)
    m = logits.max(axis=1, keepdims=True)
    ex = np.exp(logits - m)
    sm = ex / ex.sum(axis=1, keepdims=True)
    load = sm.sum(axis=0, dtype=np.float32)
    lbl = np.asarray(_cv(importance) + _cv(load), np.float32)
    util = (gates > 0).astype(np.float32).mean(axis=0, dtype=np.float32)
    return gates, top_i, lbl, util


def _split_multi_waits(mod):
    """Legalize: this walrus accepts at most one sync-wait per instruction.

    Tile's wait assigner can attach several; hoist all but the last onto
    standalone single-wait EventSemaphore instructions on the same engine,
    immediately before the original instruction (same basic block), which
    preserves the per-engine stall semantics exactly.
    """
    for fn in mod["functions"]:
        for blk in fn["blocks"]:
            new_insts = []
            for inst in blk["instructions"]:
                si = inst.get("sync_info") or {}
                waits = si.get("on_wait") or []
                if len(waits) > 1:
                    for k, w in enumerate(waits[:-1]):
                        new_insts.append(
                            {
                                "debug": inst.get("debug", 0),
                                "engine": inst["engine"],
                                "ins": [],
                                "name": f"{inst['name']}-sw{k}",
                                "opcode": "EventSemaphore",
                                "outs": [],
                                "sync_info": {"on_update": [], "on_wait": [w]},
                            }
                        )
                    si["on_wait"] = [waits[-1]]
                new_insts.append(inst)
            blk["instructions"] = new_insts
    return mod


def _patch_serializer(nc):
    orig = nc.to_json_bytes
    nc.to_json_bytes = lambda: json.dumps(_split_multi_waits(json.loads(orig()))).encode()
    return nc


def _build(C, cblocks):
    """Bass/Tile program for one core: out.T = (gelu(x@w1+b1) @ w2 + b2).T.

    DRAM layouts are pre-blocked on the host so every DMA moves [128, free]:
      xT [DBLK,128,C] bf16, w1 [DBLK,128,H] bf16, w2 [HBLK,128,D] bf16,
      b1 [HBLK,128,1] f32, b2 [DBLK,128,1] f32, out [DBLK,128,C] f32.
    """
    import concourse.bass as bass
    import concourse.mybir as mybir
    import concourse.tile as tile

    bf16, f32 = mybir.dt.bfloat16, mybir.dt.float32
    AF = mybir.ActivationFunctionType

    nc = bass.Bass()
    xT_d = nc.declare_dram_parameter("xT", [DBLK, 128, C], bf16, isOutput=False)
    w1_d = nc.declare_dram_parameter("w1", [DBLK, 128, H], bf16, isOutput=False)
    w2_d = nc.declare_dram_parameter("w2", [HBLK, 128, D], bf16, isOutput=False)
    b_d = nc.declare_dram_parameter("b", [128, HBLK + DBLK], f32, isOutput=False)
    out_d = nc.declare_dram_parameter("out", [DBLK, 128, C], f32, isOutput=True)

    WCH = 8  # w1 column chunks per d-block: PE can start after chunk 0 lands
    wch = H // WCH

    with tile.TileContext(nc) as tc:
        with (
            tc.tile_pool(name="xp", bufs=1) as xp,
            tc.tile_pool(name="w1p", bufs=1) as w1p,
            tc.tile_pool(name="w2p", bufs=1) as w2p,
            tc.tile_pool(name="bp", bufs=1) as bp,
            tc.tile_pool(name="hp", bufs=1) as hp,
            tc.tile_pool(name="op", bufs=3) as op,
            tc.tile_pool(name="ps", bufs=4, space="PSUM") as ps,
        ):
            # Load order = PE consumption order: first c-block of xT and the
            # first w1 column-chunk land first so matmuls start ~8us in, not
            # after the full 17MB of weights.
            x_sb = [
                xp.tile([128, C], bf16, tag=f"x{d}", name=f"x{d}") for d in range(DBLK)
            ]
            w1_sb = [
                w1p.tile([128, H], bf16, tag=f"w1_{d}", name=f"w1_{d}")
                for d in range(DBLK)
            ]
            c0, s0 = cblocks[0]
            for d in range(DBLK):
                nc.sync.dma_start(
                    out=x_sb[d][:, c0 : c0 + s0], in_=xT_d[d][:, c0 : c0 + s0]
                )
            for j in range(WCH):
                jsl = slice(j * wch, (j + 1) * wch)
                for d in range(DBLK):
                    nc.sync.dma_start(out=w1_sb[d][:, jsl], in_=w1_d[d][:, jsl])
                if j == 0:
                    for cstart, csize in cblocks[1:]:
                        csl = slice(cstart, cstart + csize)
                        for d in range(DBLK):
                            nc.sync.dma_start(out=x_sb[d][:, csl], in_=xT_d[d][:, csl])
            b_sb = bp.tile([128, HBLK + DBLK], f32, tag="b")
            nc.sync.dma_start(out=b_sb[:], in_=b_d[:])
            w2_sb = []
            for h in range(HBLK):
                t = w2p.tile([128, D], bf16, tag=f"w2_{h}")
                nc.sync.dma_start(out=t[:], in_=w2_d[h])
                w2_sb.append(t)

            for cstart, csize in cblocks:
                csl = slice(cstart, cstart + csize)
                # layer 1: hT[h,c] = gelu(sum_d w1[d,h].T @ xT[d,c] + b1[h])
                h_tiles = []
                for h in range(HBLK):
                    acc = ps.tile([128, csize], f32, tag="ps1")
                    for d in range(DBLK):
                        nc.tensor.matmul(
                            acc[:],
                            w1_sb[d][:, h * 128 : (h + 1) * 128],
                            x_sb[d][:, csl],
                            start=(d == 0),
                            stop=(d == DBLK - 1),
                        )
                    ht = hp.tile([128, csize], bf16, tag=f"h{h}")
                    nc.scalar.activation(ht[:], acc[:], AF.Gelu, bias=b_sb[:, h : h + 1])
                    h_tiles.append(ht)
                # layer 2: outT[d,c] = sum_h w2[h,d].T @ hT[h,c] + b2[d]
                for dd in range(DBLK):
                    acc2 = ps.tile([128, csize], f32, tag="ps2")
                    for h in range(HBLK):
                        nc.tensor.matmul(
                            acc2[:],
                            w2_sb[h][:, dd * 128 : (dd + 1) * 128],
                            h_tiles[h][:],
                            start=(h == 0),
                            stop=(h == HBLK - 1),
                        )
                    ot = op.tile([128, csize], f32, tag="ot")
                    nc.scalar.activation(
                        ot[:], acc2[:], AF.Identity, bias=b_sb[:, HBLK + dd : HBLK + dd + 1]
                    )
                    nc.sync.dma_start(out=out_d[dd][:, csl], in_=ot[:])
    return nc


def _cblocks(C):
    """Split [0, C) into near-equal blocks of at most 512 columns."""
    nb = max(1, math.ceil(C / 512))
    base = C // nb
    rem = C - base * nb
    sizes = [base + (1 if i < rem else 0) for i in range(nb)]
    blocks, s = [], 0
    for sz in sizes:
        blocks.append((s, sz))
        s += sz
    return blocks


def kernel(x, Wg, bg, w1, b1, w2, b2, training=0, **_unused):
    from concourse.bass_utils import run_bass_kernel_spmd

    x = np.asarray(x, np.float32)
    Wg = np.asarray(Wg, np.float32)
    bg = np.asarray(bg, np.float32)
    w1 = np.asarray(w1, np.float32)
    b1 = np.asarray(b1, np.float32)
    w2 = np.asarray(w2, np.float32)
    b2 = np.asarray(b2, np.float32)

    gates, top_i, lbl, util = _gate(x, Wg, bg)

    idx = [np.nonzero(gates[:, e])[0] for e in range(E)]
    counts = [len(ix) for ix in idx]
    C = max(128, max(counts))
    cblocks = _cblocks(C)

    xT_bf = x.T.astype(bfloat16)  # [D, B]
    in_maps = []
    for e in range(E):
        xg = np.zeros((D, C), bfloat16)
        xg[:, : counts[e]] = xT_bf[:, idx[e]]
        # combined per-partition bias tile: col h = b1[h*128:(h+1)*128],
        # col HBLK+d = b2[d*128:(d+1)*128]
        b_all = np.concatenate(
            [b1[e].reshape(HBLK, 128).T, b2[e].reshape(DBLK, 128).T], axis=1
        ).astype(np.float32)
        in_maps.append(
            {
                "xT": xg.reshape(DBLK, 128, C),
                "w1": w1[e].astype(bfloat16).reshape(DBLK, 128, H),
                "w2": w2[e].astype(bfloat16).reshape(HBLK, 128, D),
                "b": np.ascontiguousarray(b_all),
            }
        )

    nc = _patch_serializer(_build(C, cblocks))
    res = run_bass_kernel_spmd(
        nc,
        in_maps,
        core_ids=list(range(NCORES)),
        trace=TRACE,
        trace_cores=list(range(NCORES)) if TRACE else None,
    )
    LAST_RESULTS["exec_time_ns"] = res.exec_time_ns
    LAST_RESULTS["mean_exec_time_ns"] = res.mean_exec_time_ns
    LAST_RESULTS["res"] = res

    output = np.zeros((B, D), np.float32)
    for e in range(E):
        n = counts[e]
        if n == 0:
            continue
        oT = np.asarray(res.results[e]["out"], np.float32).reshape(D, C)
        output[idx[e]] += gates[idx[e], e][:, None] * oT[:, :n].T

    return output, gates, top_i, lbl, util


# revision 16
# speedup vs baseline: 1.1085x; 1.1085x over previous
"""MoE (B=2048, D=1024, H=4096, E=8, top-2) — Trainium2 Bass kernel, 8 NeuronCores.

Strategy (expert-parallel, sparse token routing):
  * Host: gating (x @ Wg + bg, top-2, softmax) — 0.01% of the FLOPs. The
    token->expert routing IS the sharding step: core e receives the tokens
    assigned to expert e (gathered + padded to a common capacity C).
  * Device (per core e): outT = (gelu(x_e @ w1[e] + b1[e]) @ w2[e] + b2[e]).T
    computed in bf16 with fp32 PSUM accumulation, fully transposed layout so
    no on-device transposes are needed.
  * Host: weighted scatter-combine of the two expert outputs per token.

The dense reference computes all 8 experts per token; only the top-2 survive
the gates-weighted combine, so routing does 4x less matmul work.
"""

import json
import math

import numpy as np
from ml_dtypes import bfloat16

B, D, H, E, TOP_K = 2048, 1024, 4096, 8, 2
NCORES = 8
DBLK, HBLK = D // 128, H // 128  # 8, 32

TRACE = False  # test.py sets this to capture an NTFF profile / exec_time_ns
LAST_RESULTS = {}  # test.py reads exec_time_ns etc. from here


def _gate(x, Wg, bg):
    """Mirror of the reference gating math in numpy float32."""
    logits = (x @ Wg + bg).astype(np.float32)  # [B, E]
    rows = np.arange(B)
    i1 = np.argmax(logits, axis=1)
    v1 = logits[rows, i1]
    masked = logits.copy()
    masked[rows, i1] = -np.inf
    i2 = np.argmax(masked, axis=1)
    v2 = masked[rows, i2]
    # softmax over the top-2 values (v1 >= v2)
    e2 = np.exp((v2 - v1).astype(np.float32))
    denom = (np.float32(1.0) + e2).astype(np.float32)
    g1 = (np.float32(1.0) / denom).astype(np.float32)
    g2 = (e2 / denom).astype(np.float32)
    gates = np.zeros((B, E), np.float32)
    gates[rows, i1] = g1
    gates[rows, i2] = g2
    top_i = np.stack([i1, i2], axis=1).astype(np.int32)

    # load-balance aux loss
    def _cv(v):
        return np.std(v, ddof=1).astype(np.float32) / (
            np.mean(v, dtype=np.float32) + np.float32(1e-6)
        )

    importance = gates.sum(axis=0, dtype=np.float32)
    m = logits.max(axis=1, keepdims=True)
    ex = np.exp(logits - m)
    sm = ex / ex.sum(axis=1, keepdims=True)
    load = sm.sum(axis=0, dtype=np.float32)
    lbl = np.asarray(_cv(importance) + _cv(load), np.float32)
    util = (gates > 0).astype(np.float32).mean(axis=0, dtype=np.float32)
    return gates, top_i, lbl, util


def _split_multi_waits(mod):
    """Legalize: this walrus accepts at most one sync-wait per instruction.

    Tile's wait assigner can attach several; hoist all but the last onto
    standalone single-wait EventSemaphore instructions on the same engine,
    immediately before the original instruction (same basic block), which
    preserves the per-engine stall semantics exactly.
    """
    for fn in mod["functions"]:
        for blk in fn["blocks"]:
            new_insts = []
            for inst in blk["instructions"]:
                si = inst.get("sync_info") or {}
                waits = si.get("on_wait") or []
                if len(waits) > 1:
                    for k, w in enumerate(waits[:-1]):
                        new_insts.append(
                            {
                                "debug": inst.get("debug", 0),
                                "engine": inst["engine"],
                                "ins": [],
                                "name": f"{inst['name']}-sw{k}",
                                "opcode": "EventSemaphore",
                                "outs": [],
                                "sync_info": {"on_update": [], "on_wait": [w]},
                            }
                        )
                    si["on_wait"] = [waits[-1]]
                new_insts.append(inst)
            blk["instructions"] = new_insts
    return mod


def _patch_serializer(nc):
    orig = nc.to_json_bytes
    nc.to_json_bytes = lambda: json.dumps(_split_multi_waits(json.loads(orig()))).encode()
    return nc


def _build(C, cblocks):
    """Bass/Tile program for one core: out.T = (gelu(x@w1+b1) @ w2 + b2).T.

    DRAM layouts are pre-blocked on the host so every DMA moves [128, free]:
      xT [DBLK,128,C] bf16, w1 [DBLK,128,H] bf16, w2 [HBLK,128,D] bf16,
      b1 [HBLK,128,1] f32, b2 [DBLK,128,1] f32, out [DBLK,128,C] f32.
    """
    import concourse.bass as bass
    import concourse.mybir as mybir
    import concourse.tile as tile

    bf16, f32 = mybir.dt.bfloat16, mybir.dt.float32
    AF = mybir.ActivationFunctionType

    NB = len(cblocks)
    WCH = 4  # w1 column chunks per d-block: PE can start after chunk 0 lands
    wch = H // WCH
    HPC = wch // 128  # h-blocks per w1 chunk

    nc = bass.Bass()
    # Host pre-blocks everything so each DMA is one fully contiguous block.
    xT_d = [
        nc.declare_dram_parameter(f"xT{bi}", [DBLK, 128, cs], bf16, isOutput=False)
        for bi, (_, cs) in enumerate(cblocks)
    ]
    w1_d = nc.declare_dram_parameter("w1", [WCH, DBLK, 128, wch], bf16, isOutput=False)
    w2_d = nc.declare_dram_parameter("w2", [HBLK, 128, D], bf16, isOutput=False)
    b_d = nc.declare_dram_parameter("b", [128, HBLK + DBLK], f32, isOutput=False)
    out_d = [
        nc.declare_dram_parameter(f"out{bi}", [DBLK, 128, cs], f32, isOutput=True)
        for bi, (_, cs) in enumerate(cblocks)
    ]

    with tile.TileContext(nc) as tc:
        with (
            tc.tile_pool(name="xp", bufs=1) as xp,
            tc.tile_pool(name="w1p", bufs=1) as w1p,
            tc.tile_pool(name="w2p", bufs=1) as w2p,
            tc.tile_pool(name="bp", bufs=1) as bp,
            tc.tile_pool(name="hp", bufs=1) as hp,
            tc.tile_pool(name="op", bufs=3) as op,
            tc.tile_pool(name="ps", bufs=4, space="PSUM") as ps,
        ):
            # Load order = PE consumption order. Every DMA writes a fresh tile
            # exactly once (slicing one big tile would create false WAR deps
            # that stall the later chunk loads behind early matmuls).
            # Critical-path loads (xT, w1) go on the SP HWDGE ring; bulk w2 on
            # the GpSimd SWDGE ring; outputs on the ACT ring — one sequencer
            # costs ~0.6us per DMA issue, so spreading rings matters.
            x_sb = {}  # (d, block_idx) -> [128, csize]
            w1_sb = {}  # (d, chunk_j) -> [128, wch]
            for d in range(DBLK):
                cs0 = cblocks[0][1]
                t = xp.tile([128, cs0], bf16, tag=f"x{d}_0", name=f"x{d}_0")
                nc.sync.dma_start(out=t[:], in_=xT_d[0][d])
                x_sb[(d, 0)] = t
            for j in range(WCH):
                for d in range(DBLK):
                    t = w1p.tile([128, wch], bf16, tag=f"w1_{d}_{j}", name=f"w1_{d}_{j}")
                    nc.sync.dma_start(out=t[:], in_=w1_d[j, d])
                    w1_sb[(d, j)] = t
                if j == 0:
                    for bi in range(1, NB):
                        cs = cblocks[bi][1]
                        for d in range(DBLK):
                            t = xp.tile(
                                [128, cs], bf16, tag=f"x{d}_{bi}", name=f"x{d}_{bi}"
                            )
                            nc.sync.dma_start(out=t[:], in_=xT_d[bi][d])
                            x_sb[(d, bi)] = t
            b_sb = bp.tile([128, HBLK + DBLK], f32, tag="b")
            nc.gpsimd.dma_start(out=b_sb[:], in_=b_d[:])
            w2_sb = []
            for h in range(HBLK):
                t = w2p.tile([128, D], bf16, tag=f"w2_{h}")
                nc.gpsimd.dma_start(out=t[:], in_=w2_d[h])
                w2_sb.append(t)

            for bi, (cstart, csize) in enumerate(cblocks):
                # layer 1: hT[h,c] = gelu(sum_d w1[d,h].T @ xT[d,c] + b1[h])
                h_tiles = []
                for h in range(HBLK):
                    acc = ps.tile([128, csize], f32, tag="ps1")
                    for d in range(DBLK):
                        lhsT = w1_sb[(d, h // HPC)][
                            :, (h % HPC) * 128 : (h % HPC + 1) * 128
                        ]
                        nc.tensor.matmul(
                            acc[:],
                            lhsT,
                            x_sb[(d, bi)][:],
                            start=(d == 0),
                            stop=(d == DBLK - 1),
                        )
                    ht = hp.tile([128, csize], bf16, tag=f"h{h}")
                    nc.scalar.activation(ht[:], acc[:], AF.Gelu, bias=b_sb[:, h : h + 1])
                    h_tiles.append(ht)
                # layer 2: outT[d,c] = sum_h w2[h,d].T @ hT[h,c] + b2[d]
                for dd in range(DBLK):
                    acc2 = ps.tile([128, csize], f32, tag="ps2")
                    for h in range(HBLK):
                        nc.tensor.matmul(
                            acc2[:],
                            w2_sb[h][:, dd * 128 : (dd + 1) * 128],
                            h_tiles[h][:],
                            start=(h == 0),
                            stop=(h == HBLK - 1),
                        )
                    ot = op.tile([128, csize], f32, tag="ot")
                    nc.scalar.activation(
                        ot[:], acc2[:], AF.Identity, bias=b_sb[:, HBLK + dd : HBLK + dd + 1]
                    )
                    nc.scalar.dma_start(out=out_d[bi][dd], in_=ot[:])
    return nc


def _cblocks(C):
    """Split [0, C) into near-equal blocks of at most 512 columns."""
    nb = max(1, math.ceil(C / 512))
    base = C // nb
    rem = C - base * nb
    sizes = [base + (1 if i < rem else 0) for i in range(nb)]
    blocks, s = [], 0
    for sz in sizes:
        blocks.append((s, sz))
        s += sz
    return blocks


def kernel(x, Wg, bg, w1, b1, w2, b2, training=0, **_unused):
    from concourse.bass_utils import run_bass_kernel_spmd

    x = np.asarray(x, np.float32)
    Wg = np.asarray(Wg, np.float32)
    bg = np.asarray(bg, np.float32)
    w1 = np.asarray(w1, np.float32)
    b1 = np.asarray(b1, np.float32)
    w2 = np.asarray(w2, np.float32)
    b2 = np.asarray(b2, np.float32)

    gates, top_i, lbl, util = _gate(x, Wg, bg)

    idx = [np.nonzero(gates[:, e])[0] for e in range(E)]
    counts = [len(ix) for ix in idx]
    C = max(128, max(counts))
    cblocks = _cblocks(C)

    WCH = 4
    wch = H // WCH
    xT_bf = x.T.astype(bfloat16)  # [D, B]
    in_maps = []
    for e in range(E):
        xg = np.zeros((D, C), bfloat16)
        xg[:, : counts[e]] = xT_bf[:, idx[e]]
        xg = xg.reshape(DBLK, 128, C)
        # w1 blocked chunk-major: [WCH, DBLK, 128, wch]
        w1b = np.ascontiguousarray(
            w1[e]
            .astype(bfloat16)
            .reshape(DBLK, 128, WCH, wch)
            .transpose(2, 0, 1, 3)
        )
        # combined per-partition bias tile: col h = b1[h*128:(h+1)*128],
        # col HBLK+d = b2[d*128:(d+1)*128]
        b_all = np.concatenate(
            [b1[e].reshape(HBLK, 128).T, b2[e].reshape(DBLK, 128).T], axis=1
        ).astype(np.float32)
        im = {
            "w1": w1b,
            "w2": w2[e].astype(bfloat16).reshape(HBLK, 128, D),
            "b": np.ascontiguousarray(b_all),
        }
        for bi, (cstart, csize) in enumerate(cblocks):
            im[f"xT{bi}"] = np.ascontiguousarray(xg[:, :, cstart : cstart + csize])
        in_maps.append(im)

    nc = _patch_serializer(_build(C, cblocks))
    res = run_bass_kernel_spmd(
        nc,
        in_maps,
        core_ids=list(range(NCORES)),
        trace=TRACE,
        trace_cores=list(range(NCORES)) if TRACE else None,
    )
    LAST_RESULTS["exec_time_ns"] = res.exec_time_ns
    LAST_RESULTS["mean_exec_time_ns"] = res.mean_exec_time_ns
    LAST_RESULTS["res"] = res

    output = np.zeros((B, D), np.float32)
    for e in range(E):
        n = counts[e]
        if n == 0:
            continue
        oT = np.concatenate(
            [
                np.asarray(res.results[e][f"out{bi}"], np.float32).reshape(D, cs)
                for bi, (_, cs) in enumerate(cblocks)
            ],
            axis=1,
        )
        output[idx[e]] += gates[idx[e], e][:, None] * oT[:, :n].T

    return output, gates, top_i, lbl, util
